# revision 12
# baseline (speedup 1.0000x reference)
"""Expert-parallel MoE (top-2 of 8 experts, SwiGLU FFN) for 8 Trainium2 cores.

Final structure (measured 502863 ns on HW vs 543466 ns baseline):
  - Core e holds expert e's weights; host routes/gathers (expert-parallel,
    capacity C = max expert load, bf16 operands, feature-major layout).
  - kt-outer / chunk-inner matmul ordering: consecutive matmuls share the
    stationary weight tile, which eliminates the ~40ns per-matmul
    weight-switch bubble the chunk-major ordering pays.
  - Chunks are processed in two groups (2+3) per h-tile so pg+pu
    accumulator demand stays within the 8 PSUM banks; the 2-chunk group
    goes first so the opening passes wait on 2 x-chunks, not 5.
  - x and h stay SBUF-resident; weights stream once per h-tile.
  - All DMAs are per-partition contiguous (descriptor-count-bound
    triggers); first-group x chunks arrive in two halves, low kt first.
  - Last down-projection tile runs chunk-outer so the final output
    copies/DMAs overlap the matmuls instead of draining after them.
"""

import numpy as np

DIM = 1024
HID = 2816
E = 8
TOPK = 2
P = 128
KD = DIM // P   # 8 k-subtiles (contraction of x@W)
HT = HID // P   # 22 h-subtiles
DT = DIM // P   # 8 d-subtiles (output features)
CW = 512        # chunk width cap: one PSUM bank = 512 fp32 accumulators

_KERNEL_CACHE = {}
LAST_RESULTS = None


def _chunk_sizes(C):
    """Near-equal even chunk sizes <= CW covering C (C must be even)."""
    nc_ = -(-C // CW)
    base = C // nc_
    base -= base % 2
    sizes = [base] * nc_
    extra = C - base * nc_
    i = 0
    while extra > 0:
        sizes[i] += 2
        extra -= 2
        i = (i + 1) % nc_
    assert sum(sizes) == C and all(0 < s <= CW and s % 2 == 0 for s in sizes)
    return sizes


def _chunk_groups(NC):
    """Split chunk ids into groups of <=4 so pg+pu accumulators (2 banks
    per live chunk) fit the 8 PSUM banks within one group."""
    if NC <= 4:
        return [list(range(NC))]
    h = NC // 2
    return [list(range(h)), list(range(h, NC))]


def _build_moe_ffn(sizes):
    import concourse.bass as bass  # noqa: F401
    import concourse.mybir as mybir
    from concourse import bacc, tile

    f32 = mybir.dt.float32
    bf16 = mybir.dt.bfloat16
    SiLU = mybir.ActivationFunctionType.Silu

    NC = len(sizes)
    offs = [sum(sizes[:i]) for i in range(NC)]
    groups = _chunk_groups(NC)

    nc = bacc.Bacc("TRN2", target_bir_lowering=False, debug=False)

    xt = nc.dram_tensor("xt", [P, KD * sum(sizes)], bf16, kind="ExternalInput")
    wgut = nc.dram_tensor("wgut", [HT, P, 2, KD, P], bf16,
                          kind="ExternalInput")
    wdt = nc.dram_tensor("wdt", [DT, P, HT, P], bf16, kind="ExternalInput")
    yt = nc.dram_tensor("yt", [DT, P, sum(sizes)], f32, kind="ExternalOutput")

    with tile.TileContext(nc) as tc:
        with (
            tc.tile_pool(name="xp", bufs=1) as xp,
            tc.tile_pool(name="wp", bufs=3) as wp,
            tc.tile_pool(name="hp", bufs=1) as hp,
            tc.tile_pool(name="op", bufs=4) as op,
            tc.tile_pool(name="ps", bufs=8, space="PSUM") as ps,
        ):
            w_cache = {}

            def load_w(ht):
                w_sb = wp.tile([P, 2, KD, P], bf16, tag="w", name=f"w{ht}")
                nc.sync.dma_start(w_sb[:], wgut[ht])
                w_cache[ht] = (w_sb[:, 0], w_sb[:, 1])

            # wg0 half first; wu isn't consumed until 8 passes in, so its
            # DMA goes after the first x halves (in-order transfer queue)
            wg0_sb = wp.tile([P, KD, P], bf16, tag="w0g", name="wg0")
            nc.sync.dma_start(wg0_sb[:], wgut[0, :, 0])

            # x chunk tiles; the first group's chunks are DMA'd in two
            # halves, low-kt halves first, so the opening passes start on
            # ~1MB instead of the whole 4.4MB of x
            HH = KD // 2
            x_sb = [None] * NC
            xh = {}
            wu0_sb = None
            for c, hh in [(0, 0), (1, 0), (0, 1), (1, 1), (-1, -1)]:
                if c == -1:
                    wu0_sb = wp.tile([P, KD, P], bf16, tag="w0u", name="wu0")
                    nc.sync.dma_start(wu0_sb[:], wgut[0, :, 1])
                    continue
                if c >= NC - 1:
                    continue
                n = sizes[c]
                xc = xp.tile([P, HH, n], bf16, tag=f"x{c}h{hh}",
                             name=f"x{c}h{hh}")
                base = KD * offs[c] + hh * HH * n
                nc.sync.dma_start(xc[:], xt[:, base : base + HH * n])
                xh[(c, hh)] = xc
            for c in range(NC):
                if (c, 0) in xh:
                    x_sb[c] = [xh[(c, kt // HH)][:, kt % HH]
                               for kt in range(KD)]
                else:
                    n = sizes[c]
                    xc = xp.tile([P, KD, n], bf16, tag=f"x{c}", name=f"x{c}")
                    base = KD * offs[c]
                    nc.sync.dma_start(xc[:], xt[:, base : base + KD * n])
                    x_sb[c] = [xc[:, kt] for kt in range(KD)]

            h_sb = [
                hp.tile([P, HT, sizes[c]], bf16, tag=f"h{c}", name=f"h{c}")
                for c in range(NC)
            ]

            wd_cache = {}

            def load_wd(dt):
                wd_sb = wp.tile([P, HT, P], bf16, tag="wd", bufs=2,
                                name=f"wd{dt}")
                nc.sync.dma_start(wd_sb[:], wdt[dt])
                wd_cache[dt] = wd_sb

            w_cache[0] = (wg0_sb, wu0_sb)

            # h = silu(x @ Wg) * (x @ Wu), feature-major [HID, C]
            for ht in range(HT):
                if ht + 1 < HT:
                    load_w(ht + 1)
                else:
                    load_wd(0)
                wg_sb, wu_sb = w_cache.pop(ht)
                for grp in groups:
                    pgs = {c: ps.tile([P, CW], f32, tag="acc", name=f"pg{c}")
                           for c in grp}
                    for kt in range(KD):
                        for c in grp:
                            nc.tensor.matmul(
                                pgs[c][:, : sizes[c]], wg_sb[:, kt],
                                x_sb[c][kt],
                                start=(kt == 0), stop=(kt == KD - 1),
                            )
                    sls = {}
                    for c in grp:
                        sl = op.tile([P, CW], f32, tag="sl", bufs=6, name=f"sl{c}")
                        nc.scalar.activation(
                            sl[:, : sizes[c]], pgs[c][:, : sizes[c]], SiLU)
                        sls[c] = sl
                    pus = {c: ps.tile([P, CW], f32, tag="acc", name=f"pu{c}")
                           for c in grp}
                    for kt in range(KD):
                        for c in grp:
                            nc.tensor.matmul(
                                pus[c][:, : sizes[c]], wu_sb[:, kt],
                                x_sb[c][kt],
                                start=(kt == 0), stop=(kt == KD - 1),
                            )
                    for c in grp:
                        nc.vector.tensor_mul(
                            h_sb[c][:, ht], sls[c][:, : sizes[c]],
                            pus[c][:, : sizes[c]])

            # y = h @ Wd, feature-major [DIM, C]
            for dt in range(DT):
                if dt + 1 < DT:
                    load_wd(dt + 1)
                wd_sb = wd_cache.pop(dt)
                def emit_out(c, py):
                    o_sb = op.tile([P, CW], f32, tag="o", name=f"o{c}")
                    nc.vector.tensor_copy(o_sb[:, : sizes[c]],
                                          py[:, : sizes[c]])
                    nc.sync.dma_start(
                        yt[dt, :, offs[c] : offs[c] + sizes[c]],
                        o_sb[:, : sizes[c]])

                if dt < DT - 1:
                    pys = [ps.tile([P, CW], f32, tag="acc", name=f"py{c}")
                           for c in range(NC)]
                    for ht in range(HT):
                        for c in range(NC):
                            nc.tensor.matmul(
                                pys[c][:, : sizes[c]], wd_sb[:, ht],
                                h_sb[c][:, ht],
                                start=(ht == 0), stop=(ht == HT - 1),
                            )
                    for c in range(NC):
                        emit_out(c, pys[c])
                else:
                    # last dt chunk-outer: chunks finish one at a time so
                    # the final copies+output DMAs overlap the matmuls
                    for c in range(NC):
                        py = ps.tile([P, CW], f32, tag="acc", name=f"py{c}")
                        for ht in range(HT):
                            nc.tensor.matmul(
                                py[:, : sizes[c]], wd_sb[:, ht],
                                h_sb[c][:, ht],
                                start=(ht == 0), stop=(ht == HT - 1),
                            )
                        emit_out(c, py)

    nc.finalize()
    return nc


def _get_kernel(sizes):
    key = tuple(sizes)
    if key not in _KERNEL_CACHE:
        _KERNEL_CACHE[key] = _build_moe_ffn(sizes)
    return _KERNEL_CACHE[key]


def _route(xf, W_gate):
    logits = xf.astype(np.float64) @ W_gate.astype(np.float64)  # [N, E]
    order = np.argsort(-logits, axis=1, kind="stable")[:, :TOPK]
    top = np.take_along_axis(logits, order, axis=1)
    top = top - top.max(axis=1, keepdims=True)
    ew = np.exp(top)
    w = (ew / ew.sum(axis=1, keepdims=True)).astype(np.float32)
    return order, w


def kernel(x, W_gate, Wg, Wu, Wd):
    import ml_dtypes
    from concourse.bass_utils import run_bass_kernel_spmd

    bf16 = np.dtype(ml_dtypes.bfloat16)

    x = np.ascontiguousarray(np.asarray(x, dtype=np.float32))
    W_gate = np.asarray(W_gate, dtype=np.float32)
    Wg = np.asarray(Wg, dtype=np.float32)
    Wu = np.asarray(Wu, dtype=np.float32)
    Wd = np.asarray(Wd, dtype=np.float32)

    B, T, D = x.shape
    xf = x.reshape(-1, D)
    N = xf.shape[0]

    order, w = _route(xf, W_gate)

    ids = []
    wts = []
    for e in range(E):
        sel = np.nonzero(order == e)
        ids.append(sel[0])
        wts.append(w[sel[0], sel[1]])

    max_cnt = max(len(i) for i in ids)
    C = max_cnt + (-max_cnt) % 8
    sizes = _chunk_sizes(C)
    NC = len(sizes)
    offs_h = [sum(sizes[:i]) for i in range(NC)]

    nc = _get_kernel(sizes)

    in_maps = []
    for e in range(E):
        cnt = len(ids[e])
        xe = np.zeros((C, DIM), dtype=np.float32)
        xe[:cnt] = xf[ids[e]]
        x_t = np.empty((P, KD * C), dtype=bf16)
        for c in range(NC):
            lo, n = offs_h[c], sizes[c]
            blk = xe[lo : lo + n].reshape(n, KD, P).transpose(2, 1, 0)
            x_t[:, KD * lo : KD * (lo + n)] = blk.reshape(P, KD * n).astype(bf16)
        
        wg_t = Wg[e].reshape(KD, P, HT, P).transpose(2, 1, 0, 3).astype(bf16)
        wu_t = Wu[e].reshape(KD, P, HT, P).transpose(2, 1, 0, 3).astype(bf16)
        wgu_t = np.ascontiguousarray(
            np.stack((wg_t, wu_t), axis=2))
        wd_t = np.ascontiguousarray(
            Wd[e].reshape(HT, P, DT, P).transpose(2, 1, 0, 3).astype(bf16)
        )
        in_maps.append({"xt": x_t, "wgut": wgu_t, "wdt": wd_t})

    res = run_bass_kernel_spmd(nc, in_maps, core_ids=list(range(E)))
    global LAST_RESULTS
    LAST_RESULTS = res

    out = np.zeros((N, D), dtype=np.float32)
    for e in range(E):
        cnt = len(ids[e])
        y_e = res.results[e]["yt"].reshape(DIM, C)[:, :cnt].T
        out[ids[e]] += wts[e][:, None] * y_e
    return out.reshape(B, T, D)


# revision 14
# speedup vs baseline: 1.1943x; 1.1943x over previous
"""Expert-parallel MoE (top-2 of 8 experts, SwiGLU FFN) for 8 Trainium2 cores.

Final structure (measured 502863 ns on HW vs 543466 ns baseline):
  - Core e holds expert e's weights; host routes/gathers (expert-parallel,
    capacity C = max expert load, bf16 operands, feature-major layout).
  - kt-outer / chunk-inner matmul ordering: consecutive matmuls share the
    stationary weight tile, which eliminates the ~40ns per-matmul
    weight-switch bubble the chunk-major ordering pays.
  - Chunks are processed in two groups (2+3) per h-tile so pg+pu
    accumulator demand stays within the 8 PSUM banks; the 2-chunk group
    goes first so the opening passes wait on 2 x-chunks, not 5.
  - x and h stay SBUF-resident; weights stream once per h-tile.
  - All DMAs are per-partition contiguous (descriptor-count-bound
    triggers); first-group x chunks arrive in two halves, low kt first.
  - Last down-projection tile runs chunk-outer so the final output
    copies/DMAs overlap the matmuls instead of draining after them.
"""

import numpy as np

DIM = 1024
HID = 2816
E = 8
TOPK = 2
P = 128
KD = DIM // P   # 8 k-subtiles (contraction of x@W)
HT = HID // P   # 22 h-subtiles
DT = DIM // P   # 8 d-subtiles (output features)
CW = 512        # chunk width cap: one PSUM bank = 512 fp32 accumulators

_KERNEL_CACHE = {}
LAST_RESULTS = None


def _chunk_sizes(C):
    """Near-equal even chunk sizes <= CW covering C (C must be even)."""
    nc_ = -(-C // CW)
    base = C // nc_
    base -= base % 2
    sizes = [base] * nc_
    extra = C - base * nc_
    i = 0
    while extra > 0:
        sizes[i] += 2
        extra -= 2
        i = (i + 1) % nc_
    assert sum(sizes) == C and all(0 < s <= CW and s % 2 == 0 for s in sizes)
    return sizes


def _chunk_groups(NC):
    """Split chunk ids into groups of <=4 so pg+pu accumulators (2 banks
    per live chunk) fit the 8 PSUM banks within one group."""
    if NC <= 4:
        return [list(range(NC))]
    h = NC // 2
    return [list(range(h)), list(range(h, NC))]


def _build_moe_ffn(sizes):
    import concourse.bass as bass  # noqa: F401
    import concourse.mybir as mybir
    from concourse import bacc, tile

    f32 = mybir.dt.float32
    bf16 = mybir.dt.bfloat16
    SiLU = mybir.ActivationFunctionType.Silu

    NC = len(sizes)
    offs = [sum(sizes[:i]) for i in range(NC)]
    groups = _chunk_groups(NC)

    nc = bacc.Bacc("TRN2", target_bir_lowering=False, debug=False)

    xt = nc.dram_tensor("xt", [P, KD * sum(sizes)], bf16, kind="ExternalInput")
    wgut = nc.dram_tensor("wgut", [HT, P, 2, KD, P], bf16,
                          kind="ExternalInput")
    wdt = nc.dram_tensor("wdt", [DT, P, HT, P], bf16, kind="ExternalInput")
    yt = nc.dram_tensor("yt", [DT, P, sum(sizes)], bf16,
                        kind="ExternalOutput")

    with tile.TileContext(nc) as tc:
        with (
            tc.tile_pool(name="xp", bufs=1) as xp,
            tc.tile_pool(name="wp", bufs=3) as wp,
            tc.tile_pool(name="hp", bufs=1) as hp,
            tc.tile_pool(name="op", bufs=4) as op,
            tc.tile_pool(name="ps", bufs=8, space="PSUM") as ps,
        ):
            w_cache = {}

            def load_w(ht):
                w_sb = wp.tile([P, 2, KD, P], bf16, tag="w", name=f"w{ht}")
                nc.sync.dma_start(w_sb[:], wgut[ht])
                w_cache[ht] = (w_sb[:, 0], w_sb[:, 1])

            # wg0 half first; wu isn't consumed until 8 passes in, so its
            # DMA goes after the first x halves (in-order transfer queue)
            wg0_sb = wp.tile([P, KD, P], bf16, tag="w0g", name="wg0")
            nc.sync.dma_start(wg0_sb[:], wgut[0, :, 0])

            # x chunk tiles; the first group's chunks are DMA'd in two
            # halves, low-kt halves first, so the opening passes start on
            # ~1MB instead of the whole 4.4MB of x
            HH = KD // 2
            x_sb = [None] * NC
            xh = {}
            wu0_sb = None
            for c, hh in [(0, 0), (1, 0), (-1, -1), (0, 1), (1, 1)]:
                if c == -1:
                    wu0_sb = wp.tile([P, KD, P], bf16, tag="w0u", name="wu0")
                    nc.sync.dma_start(wu0_sb[:], wgut[0, :, 1])
                    continue
                if c >= NC - 1:
                    continue
                n = sizes[c]
                xc = xp.tile([P, HH, n], bf16, tag=f"x{c}h{hh}",
                             name=f"x{c}h{hh}")
                base = KD * offs[c] + hh * HH * n
                nc.sync.dma_start(xc[:], xt[:, base : base + HH * n])
                xh[(c, hh)] = xc
            for c in range(NC):
                if (c, 0) in xh:
                    x_sb[c] = [xh[(c, kt // HH)][:, kt % HH]
                               for kt in range(KD)]
                else:
                    n = sizes[c]
                    xc = xp.tile([P, KD, n], bf16, tag=f"x{c}", name=f"x{c}")
                    base = KD * offs[c]
                    nc.sync.dma_start(xc[:], xt[:, base : base + KD * n])
                    x_sb[c] = [xc[:, kt] for kt in range(KD)]

            h_sb = [
                hp.tile([P, HT, sizes[c]], bf16, tag=f"h{c}", name=f"h{c}")
                for c in range(NC)
            ]

            wd_cache = {}

            def load_wd(dt):
                wd_sb = wp.tile([P, HT, P], bf16, tag="wd", bufs=2,
                                name=f"wd{dt}")
                nc.sync.dma_start(wd_sb[:], wdt[dt])
                wd_cache[dt] = wd_sb

            w_cache[0] = (wg0_sb, wu0_sb)

            # h = silu(x @ Wg) * (x @ Wu), feature-major [HID, C]
            for ht in range(HT):
                if ht + 1 < HT:
                    load_w(ht + 1)
                else:
                    load_wd(0)
                wg_sb, wu_sb = w_cache.pop(ht)
                for grp in groups:
                    pgs = {c: ps.tile([P, CW], f32, tag="acc", name=f"pg{c}")
                           for c in grp}
                    for kt in range(KD):
                        for c in grp:
                            nc.tensor.matmul(
                                pgs[c][:, : sizes[c]], wg_sb[:, kt],
                                x_sb[c][kt],
                                start=(kt == 0), stop=(kt == KD - 1),
                            )
                    sls = {}
                    for c in grp:
                        sl = op.tile([P, CW], f32, tag="sl", bufs=6, name=f"sl{c}")
                        nc.scalar.activation(
                            sl[:, : sizes[c]], pgs[c][:, : sizes[c]], SiLU)
                        sls[c] = sl
                    pus = {c: ps.tile([P, CW], f32, tag="acc", name=f"pu{c}")
                           for c in grp}
                    for kt in range(KD):
                        for c in grp:
                            nc.tensor.matmul(
                                pus[c][:, : sizes[c]], wu_sb[:, kt],
                                x_sb[c][kt],
                                start=(kt == 0), stop=(kt == KD - 1),
                            )
                    for c in grp:
                        nc.vector.tensor_mul(
                            h_sb[c][:, ht], sls[c][:, : sizes[c]],
                            pus[c][:, : sizes[c]])

            # y = h @ Wd, feature-major [DIM, C]
            for dt in range(DT):
                if dt + 1 < DT:
                    load_wd(dt + 1)
                wd_sb = wd_cache.pop(dt)
                def emit_out(c, py):
                    o_sb = op.tile([P, CW], bf16, tag="o", name=f"o{c}")
                    nc.vector.tensor_copy(o_sb[:, : sizes[c]],
                                          py[:, : sizes[c]])
                    nc.sync.dma_start(
                        yt[dt, :, offs[c] : offs[c] + sizes[c]],
                        o_sb[:, : sizes[c]])

                if dt < DT - 1:
                    pys = [ps.tile([P, CW], f32, tag="acc", name=f"py{c}")
                           for c in range(NC)]
                    for ht in range(HT):
                        for c in range(NC):
                            nc.tensor.matmul(
                                pys[c][:, : sizes[c]], wd_sb[:, ht],
                                h_sb[c][:, ht],
                                start=(ht == 0), stop=(ht == HT - 1),
                            )
                    for c in range(NC):
                        emit_out(c, pys[c])
                else:
                    # last dt chunk-outer: chunks finish one at a time so
                    # the final copies+output DMAs overlap the matmuls
                    for c in range(NC):
                        py = ps.tile([P, CW], f32, tag="acc", name=f"py{c}")
                        for ht in range(HT):
                            nc.tensor.matmul(
                                py[:, : sizes[c]], wd_sb[:, ht],
                                h_sb[c][:, ht],
                                start=(ht == 0), stop=(ht == HT - 1),
                            )
                        emit_out(c, py)

    nc.finalize()
    return nc


def _get_kernel(sizes):
    key = tuple(sizes)
    if key not in _KERNEL_CACHE:
        _KERNEL_CACHE[key] = _build_moe_ffn(sizes)
    return _KERNEL_CACHE[key]


def _route(xf, W_gate):
    logits = xf.astype(np.float64) @ W_gate.astype(np.float64)  # [N, E]
    order = np.argsort(-logits, axis=1, kind="stable")[:, :TOPK]
    top = np.take_along_axis(logits, order, axis=1)
    top = top - top.max(axis=1, keepdims=True)
    ew = np.exp(top)
    w = (ew / ew.sum(axis=1, keepdims=True)).astype(np.float32)
    return order, w


def kernel(x, W_gate, Wg, Wu, Wd):
    import ml_dtypes
    from concourse.bass_utils import run_bass_kernel_spmd

    bf16 = np.dtype(ml_dtypes.bfloat16)

    x = np.ascontiguousarray(np.asarray(x, dtype=np.float32))
    W_gate = np.asarray(W_gate, dtype=np.float32)
    Wg = np.asarray(Wg, dtype=np.float32)
    Wu = np.asarray(Wu, dtype=np.float32)
    Wd = np.asarray(Wd, dtype=np.float32)

    B, T, D = x.shape
    xf = x.reshape(-1, D)
    N = xf.shape[0]

    order, w = _route(xf, W_gate)

    ids = []
    wts = []
    for e in range(E):
        sel = np.nonzero(order == e)
        ids.append(sel[0])
        wts.append(w[sel[0], sel[1]])

    max_cnt = max(len(i) for i in ids)
    C = max_cnt + (-max_cnt) % 8
    sizes = _chunk_sizes(C)
    NC = len(sizes)
    offs_h = [sum(sizes[:i]) for i in range(NC)]

    nc = _get_kernel(sizes)

    in_maps = []
    for e in range(E):
        cnt = len(ids[e])
        xe = np.zeros((C, DIM), dtype=np.float32)
        xe[:cnt] = xf[ids[e]]
        x_t = np.empty((P, KD * C), dtype=bf16)
        for c in range(NC):
            lo, n = offs_h[c], sizes[c]
            blk = xe[lo : lo + n].reshape(n, KD, P).transpose(2, 1, 0)
            x_t[:, KD * lo : KD * (lo + n)] = blk.reshape(P, KD * n).astype(bf16)
        
        wg_t = Wg[e].reshape(KD, P, HT, P).transpose(2, 1, 0, 3).astype(bf16)
        wu_t = Wu[e].reshape(KD, P, HT, P).transpose(2, 1, 0, 3).astype(bf16)
        wgu_t = np.ascontiguousarray(
            np.stack((wg_t, wu_t), axis=2))
        wd_t = np.ascontiguousarray(
            Wd[e].reshape(HT, P, DT, P).transpose(2, 1, 0, 3).astype(bf16)
        )
        in_maps.append({"xt": x_t, "wgut": wgu_t, "wdt": wd_t})

    res = run_bass_kernel_spmd(nc, in_maps, core_ids=list(range(E)))
    global LAST_RESULTS
    LAST_RESULTS = res

    out = np.zeros((N, D), dtype=np.float32)
    for e in range(E):
        cnt = len(ids[e])
        y_e = res.results[e]["yt"].astype(np.float32)\
            .reshape(DIM, C)[:, :cnt].T
        out[ids[e]] += wts[e][:, None] * y_e
    return out.reshape(B, T, D)


# revision 15
# speedup vs baseline: 1.1987x; 1.0037x over previous
"""Expert-parallel MoE (top-2 of 8 experts, SwiGLU FFN) for 8 Trainium2 cores.

Final structure (measured 503985 ns on HW vs 543466 ns baseline;
best sibling config measured 502863):
  - Core e holds expert e's weights; host routes/gathers (expert-parallel,
    capacity C = max expert load, bf16 operands, feature-major layout).
  - kt-outer / chunk-inner matmul ordering: consecutive matmuls share the
    stationary weight tile, which eliminates the ~40ns per-matmul
    weight-switch bubble the chunk-major ordering pays.
  - Chunks are processed in two groups (2+3) per h-tile so pg+pu
    accumulator demand stays within the 8 PSUM banks; the 2-chunk group
    goes first so the opening passes wait on 2 x-chunks, not 5.
  - x and h stay SBUF-resident; weights stream once per h-tile.
  - All DMAs are per-partition contiguous (descriptor-count-bound
    triggers); first-group x chunks arrive in two halves, low kt first.
  - Last down-projection tile runs chunk-outer so the final output
    copies/DMAs overlap the matmuls instead of draining after them.
  - Output is written back in bf16 (halves the 8.8MB y write-back;
    host combine upcasts to fp32; rel err 4.9e-3 vs the 2e-2 gate).
"""

import numpy as np

DIM = 1024
HID = 2816
E = 8
TOPK = 2
P = 128
KD = DIM // P   # 8 k-subtiles (contraction of x@W)
HT = HID // P   # 22 h-subtiles
DT = DIM // P   # 8 d-subtiles (output features)
CW = 512        # chunk width cap: one PSUM bank = 512 fp32 accumulators

_KERNEL_CACHE = {}
LAST_RESULTS = None


def _chunk_sizes(C):
    """Near-equal even chunk sizes <= CW covering C (C must be even)."""
    nc_ = -(-C // CW)
    base = C // nc_
    base -= base % 2
    sizes = [base] * nc_
    extra = C - base * nc_
    i = 0
    while extra > 0:
        sizes[i] += 2
        extra -= 2
        i = (i + 1) % nc_
    assert sum(sizes) == C and all(0 < s <= CW and s % 2 == 0 for s in sizes)
    return sizes


def _chunk_groups(NC):
    """Split chunk ids into groups of <=4 so pg+pu accumulators (2 banks
    per live chunk) fit the 8 PSUM banks within one group."""
    if NC <= 4:
        return [list(range(NC))]
    h = NC // 2
    return [list(range(h)), list(range(h, NC))]


def _build_moe_ffn(sizes):
    import concourse.bass as bass  # noqa: F401
    import concourse.mybir as mybir
    from concourse import bacc, tile

    f32 = mybir.dt.float32
    bf16 = mybir.dt.bfloat16
    SiLU = mybir.ActivationFunctionType.Silu

    NC = len(sizes)
    offs = [sum(sizes[:i]) for i in range(NC)]
    groups = _chunk_groups(NC)

    nc = bacc.Bacc("TRN2", target_bir_lowering=False, debug=False)

    xt = nc.dram_tensor("xt", [P, KD * sum(sizes)], bf16, kind="ExternalInput")
    wgut = nc.dram_tensor("wgut", [HT, P, 2, KD, P], bf16,
                          kind="ExternalInput")
    wdt = nc.dram_tensor("wdt", [DT, P, HT, P], bf16, kind="ExternalInput")
    yt = nc.dram_tensor("yt", [DT, P, sum(sizes)], bf16,
                        kind="ExternalOutput")

    with tile.TileContext(nc) as tc:
        with (
            tc.tile_pool(name="xp", bufs=1) as xp,
            tc.tile_pool(name="wp", bufs=3) as wp,
            tc.tile_pool(name="hp", bufs=1) as hp,
            tc.tile_pool(name="op", bufs=4) as op,
            tc.tile_pool(name="ps", bufs=8, space="PSUM") as ps,
        ):
            w_cache = {}

            def load_w(ht):
                w_sb = wp.tile([P, 2, KD, P], bf16, tag="w", name=f"w{ht}")
                nc.sync.dma_start(w_sb[:], wgut[ht])
                w_cache[ht] = (w_sb[:, 0], w_sb[:, 1])

            # wg0 half first; wu isn't consumed until 8 passes in, so its
            # DMA goes after the first x halves (in-order transfer queue)
            wg0_sb = wp.tile([P, KD, P], bf16, tag="w0g", name="wg0")
            nc.sync.dma_start(wg0_sb[:], wgut[0, :, 0])

            # x chunk tiles; the first group's chunks are DMA'd in two
            # halves, low-kt halves first, so the opening passes start on
            # ~1MB instead of the whole 4.4MB of x
            HH = KD // 2
            x_sb = [None] * NC
            xh = {}
            wu0_sb = None
            for c, hh in [(0, 0), (1, 0), (-1, -1), (0, 1), (1, 1)]:
                if c == -1:
                    wu0_sb = wp.tile([P, KD, P], bf16, tag="w0u", name="wu0")
                    nc.sync.dma_start(wu0_sb[:], wgut[0, :, 1])
                    continue
                if c >= NC - 1:
                    continue
                n = sizes[c]
                xc = xp.tile([P, HH, n], bf16, tag=f"x{c}h{hh}",
                             name=f"x{c}h{hh}")
                base = KD * offs[c] + hh * HH * n
                nc.sync.dma_start(xc[:], xt[:, base : base + HH * n])
                xh[(c, hh)] = xc
            for c in range(NC):
                if (c, 0) in xh:
                    x_sb[c] = [xh[(c, kt // HH)][:, kt % HH]
                               for kt in range(KD)]
                else:
                    n = sizes[c]
                    xc = xp.tile([P, KD, n], bf16, tag=f"x{c}", name=f"x{c}")
                    base = KD * offs[c]
                    nc.sync.dma_start(xc[:], xt[:, base : base + KD * n])
                    x_sb[c] = [xc[:, kt] for kt in range(KD)]

            h_sb = [
                hp.tile([P, HT, sizes[c]], bf16, tag=f"h{c}", name=f"h{c}")
                for c in range(NC)
            ]

            wd_cache = {}

            def load_wd(dt):
                wd_sb = wp.tile([P, HT, P], bf16, tag="wd", bufs=2,
                                name=f"wd{dt}")
                nc.sync.dma_start(wd_sb[:], wdt[dt])
                wd_cache[dt] = wd_sb

            w_cache[0] = (wg0_sb, wu0_sb)

            # h = silu(x @ Wg) * (x @ Wu), feature-major [HID, C]
            for ht in range(HT):
                if ht + 1 < HT:
                    load_w(ht + 1)
                else:
                    load_wd(0)
                wg_sb, wu_sb = w_cache.pop(ht)
                for grp in groups:
                    pgs = {c: ps.tile([P, CW], f32, tag="acc", name=f"pg{c}")
                           for c in grp}
                    for kt in range(KD):
                        for c in grp:
                            nc.tensor.matmul(
                                pgs[c][:, : sizes[c]], wg_sb[:, kt],
                                x_sb[c][kt],
                                start=(kt == 0), stop=(kt == KD - 1),
                            )
                    sls = {}
                    for c in grp:
                        sl = op.tile([P, CW], f32, tag="sl", bufs=6, name=f"sl{c}")
                        nc.scalar.activation(
                            sl[:, : sizes[c]], pgs[c][:, : sizes[c]], SiLU)
                        sls[c] = sl
                    pus = {c: ps.tile([P, CW], f32, tag="acc", name=f"pu{c}")
                           for c in grp}
                    for kt in range(KD):
                        for c in grp:
                            nc.tensor.matmul(
                                pus[c][:, : sizes[c]], wu_sb[:, kt],
                                x_sb[c][kt],
                                start=(kt == 0), stop=(kt == KD - 1),
                            )
                    for c in grp:
                        nc.vector.tensor_mul(
                            h_sb[c][:, ht], sls[c][:, : sizes[c]],
                            pus[c][:, : sizes[c]])

            # y = h @ Wd, feature-major [DIM, C]
            for dt in range(DT):
                if dt + 1 < DT:
                    load_wd(dt + 1)
                wd_sb = wd_cache.pop(dt)
                def emit_out(c, py):
                    o_sb = op.tile([P, CW], bf16, tag="o", name=f"o{c}")
                    nc.vector.tensor_copy(o_sb[:, : sizes[c]],
                                          py[:, : sizes[c]])
                    nc.sync.dma_start(
                        yt[dt, :, offs[c] : offs[c] + sizes[c]],
                        o_sb[:, : sizes[c]])

                if dt < DT - 1:
                    pys = [ps.tile([P, CW], f32, tag="acc", name=f"py{c}")
                           for c in range(NC)]
                    for ht in range(HT):
                        for c in range(NC):
                            nc.tensor.matmul(
                                pys[c][:, : sizes[c]], wd_sb[:, ht],
                                h_sb[c][:, ht],
                                start=(ht == 0), stop=(ht == HT - 1),
                            )
                    for c in range(NC):
                        emit_out(c, pys[c])
                else:
                    # last dt chunk-outer: chunks finish one at a time so
                    # the final copies+output DMAs overlap the matmuls
                    for c in range(NC):
                        py = ps.tile([P, CW], f32, tag="acc", name=f"py{c}")
                        for ht in range(HT):
                            nc.tensor.matmul(
                                py[:, : sizes[c]], wd_sb[:, ht],
                                h_sb[c][:, ht],
                                start=(ht == 0), stop=(ht == HT - 1),
                            )
                        emit_out(c, py)

    nc.finalize()
    return nc


def _get_kernel(sizes):
    key = tuple(sizes)
    if key not in _KERNEL_CACHE:
        _KERNEL_CACHE[key] = _build_moe_ffn(sizes)
    return _KERNEL_CACHE[key]


def _route(xf, W_gate):
    logits = xf.astype(np.float64) @ W_gate.astype(np.float64)  # [N, E]
    order = np.argsort(-logits, axis=1, kind="stable")[:, :TOPK]
    top = np.take_along_axis(logits, order, axis=1)
    top = top - top.max(axis=1, keepdims=True)
    ew = np.exp(top)
    w = (ew / ew.sum(axis=1, keepdims=True)).astype(np.float32)
    return order, w


def kernel(x, W_gate, Wg, Wu, Wd):
    import ml_dtypes
    from concourse.bass_utils import run_bass_kernel_spmd

    bf16 = np.dtype(ml_dtypes.bfloat16)

    x = np.ascontiguousarray(np.asarray(x, dtype=np.float32))
    W_gate = np.asarray(W_gate, dtype=np.float32)
    Wg = np.asarray(Wg, dtype=np.float32)
    Wu = np.asarray(Wu, dtype=np.float32)
    Wd = np.asarray(Wd, dtype=np.float32)

    B, T, D = x.shape
    xf = x.reshape(-1, D)
    N = xf.shape[0]

    order, w = _route(xf, W_gate)

    ids = []
    wts = []
    for e in range(E):
        sel = np.nonzero(order == e)
        ids.append(sel[0])
        wts.append(w[sel[0], sel[1]])

    max_cnt = max(len(i) for i in ids)
    C = max_cnt + (-max_cnt) % 8
    sizes = _chunk_sizes(C)
    NC = len(sizes)
    offs_h = [sum(sizes[:i]) for i in range(NC)]

    nc = _get_kernel(sizes)

    in_maps = []
    for e in range(E):
        cnt = len(ids[e])
        xe = np.zeros((C, DIM), dtype=np.float32)
        xe[:cnt] = xf[ids[e]]
        x_t = np.empty((P, KD * C), dtype=bf16)
        for c in range(NC):
            lo, n = offs_h[c], sizes[c]
            blk = xe[lo : lo + n].reshape(n, KD, P).transpose(2, 1, 0)
            x_t[:, KD * lo : KD * (lo + n)] = blk.reshape(P, KD * n).astype(bf16)
        
        wg_t = Wg[e].reshape(KD, P, HT, P).transpose(2, 1, 0, 3).astype(bf16)
        wu_t = Wu[e].reshape(KD, P, HT, P).transpose(2, 1, 0, 3).astype(bf16)
        wgu_t = np.ascontiguousarray(
            np.stack((wg_t, wu_t), axis=2))
        wd_t = np.ascontiguousarray(
            Wd[e].reshape(HT, P, DT, P).transpose(2, 1, 0, 3).astype(bf16)
        )
        in_maps.append({"xt": x_t, "wgut": wgu_t, "wdt": wd_t})

    res = run_bass_kernel_spmd(nc, in_maps, core_ids=list(range(E)))
    global LAST_RESULTS
    LAST_RESULTS = res

    out = np.zeros((N, D), dtype=np.float32)
    for e in range(E):
        cnt = len(ids[e])
        y_e = res.results[e]["yt"].astype(np.float32)\
            .reshape(DIM, C)[:, :cnt].T
        out[ids[e]] += wts[e][:, None] * y_e
    return out.reshape(B, T, D)
